# revision 31
# baseline (speedup 1.0000x reference)
"""Causal self-attention (B=2, T=2048, C=1024, H=16, rope) on 8 trn2 cores.

Sharding: core i = (batch b = i // 4, head-group g = i % 4 owning heads 4g..4g+3).
Each core computes its 4 heads' attention and a partial projection (transposed,
bf16); the host sums the 4 head-group partials per batch and adds the biases
(v-bias is folded into a host-side constant since sum(att) == 1).

All matmul operands are bf16 (1 cycle/row at any N). x arrives transposed from
the host; rope is 6 tensor ops on DVE (2x bf16 mode); qk bias is a per-partition
tensor_scalar_add fused into the psum evict; exp pairs on Act; causal mask on
DVE; denominators via a ones column in v (PV matmul row 64); 1/l broadcast via
gpsimd partition_broadcast.

Emission order pipelines chunks: attention(t) -> phase_a(t+1) -> proj(t), so
the softmax-normalize chain and proj of chunk t hide behind the qkv matmuls of
chunk t+1.
"""

import numpy as np

B, T, C, H = 2, 2048, 1024, 16
HS = C // H            # 64
HPC = H // 4           # 4 heads per core
NCORES = 8
TCH = 512              # t/q chunk size
NCH = T // TCH         # 4 chunks
NSLAB = T // 128       # 16 t-slabs

_cache = {}
last_results = None    # BassKernelResults of the most recent run (for test.py)


def _build():
    import concourse.bacc as bacc
    import concourse.mybir as mybir
    import concourse.tile as tile

    F32 = mybir.dt.float32
    F32R = mybir.dt.float32r
    BF16 = mybir.dt.bfloat16
    AF = mybir.ActivationFunctionType

    nc = bacc.Bacc("TRN2", target_bir_lowering=False, debug=False,
                   num_devices=NCORES)

    xt_in = nc.dram_tensor("xt_in", (C, T), BF16, kind="ExternalInput")
    wqk = nc.dram_tensor("wqk", (C, 512), BF16, kind="ExternalInput")
    bqk_c = nc.dram_tensor("bqk_c", (128, 4), F32, kind="ExternalInput")
    wv = nc.dram_tensor("wv", (C, 256), BF16, kind="ExternalInput")
    wp = nc.dram_tensor("wp", (256, C), BF16, kind="ExternalInput")
    cos_in = nc.dram_tensor("cos_in", (128, T), BF16, kind="ExternalInput")
    sin_in = nc.dram_tensor("sin_in", (128, T), BF16, kind="ExternalInput")
    cmask = nc.dram_tensor("cmask", (128, 128), BF16, kind="ExternalInput")
    out_t = nc.dram_tensor("out_t", (C, T), BF16, kind="ExternalOutput")

    with tile.TileContext(nc) as tc:
        with (
            tc.tile_pool(name="const", bufs=1) as const,
            tc.tile_pool(name="xp", bufs=2) as xp,
            tc.tile_pool(name="work", bufs=3) as work,
            tc.tile_pool(name="ep", bufs=4) as ep,
            tc.tile_pool(name="yp", bufs=2) as yp,
            tc.tile_pool(name="ost", bufs=2) as ost,
            tc.tile_pool(name="ps_a", bufs=2, space="PSUM") as ps_a,
            tc.tile_pool(name="ps_s", bufs=2, space="PSUM") as ps_s,
            tc.tile_pool(name="ps_o", bufs=2, space="PSUM") as ps_o,
        ):
            # ---- chunk-0 x as per-slab DMAs so the first qk matmuls can
            # start as soon as slab 0 + the first wqk m-tile land ----
            xts = [xp.tile([128, 8, TCH], BF16, tag="xt", name=f"xt{c}")
                   for c in range(NCH)]
            for s in range(8):
                eng = nc.sync if s % 2 == 0 else nc.gpsimd
                eng.dma_start(
                    xts[0][:, s, :],
                    xt_in.ap()[s * 128:(s + 1) * 128, 0:TCH])
            # m-tile-major so the first qk matmul only waits on one small DMA
            wqk_sb = const.tile([128, 4, 8, 128], BF16)
            for m in range(4):
                nc.scalar.dma_start(
                    wqk_sb[:, m, :, :],
                    wqk.ap()[:, m * 128:(m + 1) * 128]
                    .rearrange("(s p) c -> p s c", p=128))
            bqk_sb = const.tile([128, 4], F32)
            nc.gpsimd.dma_start(bqk_sb[:], bqk_c[:, :])
            cos_sb = const.tile([128, T], BF16)
            nc.gpsimd.dma_start(cos_sb[:], cos_in[:, :])
            sin_sb = const.tile([128, T], BF16)
            nc.gpsimd.dma_start(sin_sb[:], sin_in[:, :])
            wv_sb = const.tile([128, 8, 256], BF16)
            nc.sync.dma_start(wv_sb[:],
                              wv.ap().rearrange("(s p) m -> p s m", p=128))
            msk_sb = const.tile([128, 128], BF16)
            nc.gpsimd.dma_start(msk_sb[:], cmask[:, :])
            wp_sb = const.tile([128, 2, C], BF16)
            nc.scalar.dma_start(wp_sb[:],
                                wp.ap().rearrange("(s p) m -> p s m", p=128))

            # ---- persistent activations ----
            qT = [const.tile([128, T], BF16, name=f"qT{p}", tag=f"qT{p}")
                  for p in range(2)]
            kT = [const.tile([128, T], BF16, name=f"kT{p}", tag=f"kT{p}")
                  for p in range(2)]
            # v with ones column: [t-slab-part, slab, head, 65]
            v_sb = const.tile([128, NSLAB, HPC, 65], BF16)
            ones128 = const.tile([128, 64], F32)
            nc.gpsimd.memset(ones128[:], 1.0)
            nc.vector.tensor_copy(
                v_sb[:, :, :, 64],
                ones128[:, 0:64].rearrange("p (s h) -> p s h", s=NSLAB))

            swap = [(0, 32, 32, 64), (32, 64, 0, 32),
                    (64, 96, 96, 128), (96, 128, 64, 96)]

            def phase_a(tcH):
                """qk matmuls + rope + v matmuls for chunk tcH."""
                tcols = slice(tcH * TCH, (tcH + 1) * TCH)
                xt_ch = xts[tcH]
                for m in range(4):
                    pqk = ps_a.tile([128, TCH], F32, tag="a")
                    for s in range(8):
                        nc.tensor.matmul(pqk[:], wqk_sb[:, m, s, :],
                                         xt_ch[:, s, :], start=(s == 0),
                                         stop=(s == 7))
                    tQr = work.tile([128, TCH], BF16, tag="tQr")
                    nc.vector.tensor_scalar_add(tQr[:], pqk[:],
                                                bqk_sb[:, m:m + 1])
                    tQc = work.tile([128, TCH], BF16, tag="tQc")
                    nc.vector.tensor_mul(tQc[:], tQr[:], cos_sb[:, tcols])
                    tQs = work.tile([128, TCH], BF16, tag="tQs")
                    for (a0, a1, b0, b1) in swap:
                        nc.vector.tensor_mul(tQs[a0:a1, :], tQr[b0:b1, :],
                                             sin_sb[b0:b1, tcols])
                    dest = (qT if m % 2 == 0 else kT)[m // 2]
                    nc.vector.tensor_add(dest[:, tcols], tQc[:], tQs[:])

                for ts in range(4):
                    pv = ps_a.tile([128, 256], F32, tag="a")
                    for s in range(8):
                        nc.tensor.matmul(pv[:], xt_ch[:, s, ts * 128:(ts + 1) * 128],
                                         wv_sb[:, s, :], start=(s == 0),
                                         stop=(s == 7))
                    sl = tcH * 4 + ts
                    nc.scalar.activation(
                        v_sb[:, sl, :, 0:64],
                        pv[:].rearrange("p (h e) -> p h e", e=64), AF.Copy)

            def attention(tcH):
                """Returns the normalized yT tile for chunk tcH."""
                tcols = slice(tcH * TCH, (tcH + 1) * TCH)
                # prefetch next x chunk early (bus is quiet during attention)
                if tcH + 1 < NCH:
                    nxt = slice((tcH + 1) * TCH, (tcH + 2) * TCH)
                    nc.sync.dma_start(
                        xts[tcH + 1][:],
                        xt_in.ap()[:, nxt].rearrange("(s p) m -> p s m", p=128))

                yT_ch = yp.tile([128, 2, TCH], BF16, tag="yT")
                nslabs = 4 * tcH + 4

                for p in range(2):
                    pos = [ps_o.tile([128, TCH], F32, tag="O", name=f"po{hh}")
                           for hh in range(2)]

                    def emit_S(j):
                        rr = j - 4 * tcH
                        r = max(rr, 0) * 128
                        qs = slice(tcH * TCH + r, (tcH + 1) * TCH)
                        psS = ps_s.tile([128, 2, TCH], F32, tag="S")
                        for hh in range(2):
                            base = 64 * hh
                            nc.tensor.matmul(
                                psS[:, hh, r:TCH],
                                kT[p][base:base + 64, j * 128:(j + 1) * 128],
                                qT[p][base:base + 64, qs],
                                start=True, stop=True)
                        expS = ep.tile([128, 2, TCH], BF16, tag="expS")
                        nc.scalar.activation(expS[:, :, r:TCH], psS[:, :, r:TCH],
                                             AF.Exp, scale=0.125)
                        if rr >= 0:
                            for hh in range(2):
                                nc.vector.tensor_mul(expS[:, hh, r:r + 128],
                                                     expS[:, hh, r:r + 128],
                                                     msk_sb[:, :])
                        return expS, r

                    def emit_PV(j, expS, r, hh):
                        h = 2 * p + hh
                        nc.tensor.matmul(pos[hh][0:65, r:TCH],
                                         v_sb[:, j, h, :],
                                         expS[:, hh, r:TCH],
                                         start=(j == 0),
                                         stop=(j == nslabs - 1))

                    def emit_norm(hh):
                        base, po = 64 * hh, pos[hh]
                        l_r = work.tile([1, TCH], F32R, tag="lr")
                        with nc.allow_low_precision(reason="f32r rounding of 1/l"):
                            nc.vector.reciprocal(l_r[:], po[64:65, :])
                        lbc = work.tile([64, TCH], F32R, tag="lbc")
                        nc.gpsimd.partition_broadcast(lbc[:], l_r[:])
                        nc.vector.tensor_mul(yT_ch[base:base + 64, p, :],
                                             po[0:64, :], lbc[:])

                    # software pipeline depth 2: S(j+2) issued before PV(j) so
                    # exp latency and the po handover hide behind PE work
                    win = [emit_S(0)]
                    if nslabs > 1:
                        win.append(emit_S(1))
                    for j in range(nslabs):
                        if j + 2 < nslabs:
                            win.append(emit_S(j + 2))
                        expS, r = win[0]
                        if j == nslabs - 1:
                            emit_PV(j, expS, r, 0)
                            emit_norm(0)
                            emit_PV(j, expS, r, 1)
                            emit_norm(1)
                        else:
                            emit_PV(j, expS, r, 0)
                            emit_PV(j, expS, r, 1)
                        win.pop(0)
                return yT_ch

            def proj(tcH, yT_ch):
                tcols = slice(tcH * TCH, (tcH + 1) * TCH)
                for mp in range(4):
                    pp = ps_s.tile([128, 2, TCH], F32, tag="S")
                    for half in range(2):
                        mo = 2 * mp + half
                        for s in range(2):
                            nc.tensor.matmul(pp[:, half, :],
                                             wp_sb[:, s, mo * 128:(mo + 1) * 128],
                                             yT_ch[:, s, :], start=(s == 0),
                                             stop=(s == 1))
                    o_st = ost.tile([128, 2, TCH], BF16, tag="ost")
                    if mp % 2 == 0:
                        nc.vector.tensor_copy(o_st[:], pp[:])
                    else:
                        nc.scalar.activation(o_st[:], pp[:], AF.Copy)
                    for half in range(2):
                        mo = 2 * mp + half
                        nc.sync.dma_start(out_t[mo * 128:(mo + 1) * 128, tcols],
                                          o_st[:, half, :])

            # pipeline: attention(t) -> phase_a(t+1) -> proj(t)
            phase_a(0)
            for tcH in range(NCH):
                yT_ch = attention(tcH)
                if tcH + 1 < NCH:
                    phase_a(tcH + 1)
                proj(tcH, yT_ch)

    nc.compile()
    return nc


def _rope_tables():
    pos = np.arange(T, dtype=np.float32)[:, None]                  # [T, 1]
    i = np.arange(1, HS // 2 + 1, dtype=np.float32)[None]          # [1, 32]
    theta = 1.0 / 10000.0 ** (2.0 * (i - 1.0) / HS)
    ang = pos * theta
    cos, sin = np.cos(ang).T, np.sin(ang).T                        # [32, T]
    cos_rep = np.tile(cos, (4, 1)).astype(np.float32)              # [128, T]
    sin_sgn = np.concatenate([sin, -sin, sin, -sin], 0).astype(np.float32)
    return cos_rep, sin_sgn


def _mask128():
    p = np.arange(128)[:, None]
    f = np.arange(128)[None, :]
    return (p <= f).astype(np.float32)


def kernel(x, W_qkv, b_qkv, W_proj, b_proj):
    global last_results
    import ml_dtypes
    from concourse.bass_utils import run_bass_kernel_spmd

    bf16 = ml_dtypes.bfloat16

    if "nc" not in _cache:
        _cache["nc"] = _build()
    nc = _cache["nc"]

    x = np.asarray(x, np.float32)
    W_qkv = np.asarray(W_qkv, np.float32)
    b_qkv = np.asarray(b_qkv, np.float32)
    W_proj = np.asarray(W_proj, np.float32)
    b_proj = np.asarray(b_proj, np.float32)

    perm = np.concatenate([np.arange(0, HS, 2), np.arange(1, HS, 2)])  # even|odd
    cos_rep, sin_sgn = _rope_tables()
    cmask = _mask128()

    in_maps = []
    for core in range(NCORES):
        b, g = core // 4, core % 4
        heads = [4 * g + j for j in range(HPC)]
        wq = [W_qkv[:, h * 3 * HS:h * 3 * HS + HS][:, perm] for h in heads]
        wk = [W_qkv[:, h * 3 * HS + HS:h * 3 * HS + 2 * HS][:, perm] for h in heads]
        wv_ = [W_qkv[:, h * 3 * HS + 2 * HS:h * 3 * HS + 3 * HS] for h in heads]
        bq = [b_qkv[h * 3 * HS:h * 3 * HS + HS][perm] for h in heads]
        bk = [b_qkv[h * 3 * HS + HS:h * 3 * HS + 2 * HS][perm] for h in heads]
        # col-chunks: [q01 | k01 | q23 | k23]
        wqk = np.concatenate([wq[0], wq[1], wk[0], wk[1],
                              wq[2], wq[3], wk[2], wk[3]], axis=1)
        bqk = np.concatenate([bq[0], bq[1], bk[0], bk[1],
                              bq[2], bq[3], bk[2], bk[3]])
        in_maps.append({
            "xt_in": np.ascontiguousarray(x[b].T).astype(bf16),
            "wqk": np.ascontiguousarray(wqk).astype(bf16),
            "bqk_c": np.ascontiguousarray(bqk.reshape(4, 128).T),
            "wv": np.ascontiguousarray(np.concatenate(wv_, axis=1)).astype(bf16),
            "wp": np.ascontiguousarray(W_proj[g * 256:(g + 1) * 256, :]).astype(bf16),
            "cos_in": cos_rep.astype(bf16),
            "sin_in": sin_sgn.astype(bf16),
            "cmask": cmask.astype(bf16),
        })

    res = run_bass_kernel_spmd(nc, in_maps, core_ids=list(range(NCORES)))
    last_results = res

    out = np.zeros((B, T, C), dtype=np.float32)
    for core in range(NCORES):
        b = core // 4
        out[b] += res.results[core]["out_t"].astype(np.float32).T
    # v-bias shifts y by exactly bv per head (sum(att) == 1), so its effect
    # on the output is the constant bv_full @ W_proj
    bv_full = np.concatenate(
        [b_qkv[h * 3 * HS + 2 * HS:h * 3 * HS + 3 * HS] for h in range(H)])
    out += (b_proj + bv_full @ W_proj)[None, None, :]
    return out


# revision 33
# speedup vs baseline: 1.0897x; 1.0897x over previous
"""Causal self-attention (B=2, T=2048, C=1024, H=16, rope) on 8 trn2 cores.

Sharding: core i = (batch b = i // 4, head-group g = i % 4 owning heads 4g..4g+3).
Each core computes its 4 heads' attention and a partial projection (transposed,
bf16); the host sums the 4 head-group partials per batch and adds the biases
(v-bias is folded into a host-side constant since sum(att) == 1).

All matmul operands are bf16 (1 cycle/row at any N). x arrives transposed from
the host; rope is 6 tensor ops on DVE (2x bf16 mode); qk bias is a per-partition
tensor_scalar_add fused into the psum evict; exp pairs on Act; causal mask on
DVE; denominators via a ones column in v (PV matmul row 64); 1/l broadcast via
gpsimd partition_broadcast.

Emission WEAVES work at instruction granularity: the attention j-loop of chunk
t is exp(Act)-paced, leaving PE micro-gaps; units of phase A(t+1) (qk m-tiles,
v tiles) and proj(t-1) (mo tiles) are emitted between j iterations so PE's
program order fills those gaps and the normalize chain never blocks proj.
"""

import numpy as np

B, T, C, H = 2, 2048, 1024, 16
HS = C // H            # 64
HPC = H // 4           # 4 heads per core
NCORES = 8
TCH = 512              # t/q chunk size
NCH = T // TCH         # 4 chunks
NSLAB = T // 128       # 16 t-slabs

_cache = {}
last_results = None    # BassKernelResults of the most recent run (for test.py)


def _build():
    import concourse.bacc as bacc
    import concourse.mybir as mybir
    import concourse.tile as tile

    F32 = mybir.dt.float32
    F32R = mybir.dt.float32r
    BF16 = mybir.dt.bfloat16
    AF = mybir.ActivationFunctionType

    nc = bacc.Bacc("TRN2", target_bir_lowering=False, debug=False,
                   num_devices=NCORES)

    xt_in = nc.dram_tensor("xt_in", (C, T), BF16, kind="ExternalInput")
    wqk = nc.dram_tensor("wqk", (C, 512), BF16, kind="ExternalInput")
    bqk_c = nc.dram_tensor("bqk_c", (128, 4), F32, kind="ExternalInput")
    wv = nc.dram_tensor("wv", (C, 256), BF16, kind="ExternalInput")
    wp = nc.dram_tensor("wp", (256, C), BF16, kind="ExternalInput")
    cos_in = nc.dram_tensor("cos_in", (128, T), BF16, kind="ExternalInput")
    sin_in = nc.dram_tensor("sin_in", (128, T), BF16, kind="ExternalInput")
    cmask = nc.dram_tensor("cmask", (128, 128), BF16, kind="ExternalInput")
    out_t = nc.dram_tensor("out_t", (C, T), BF16, kind="ExternalOutput")

    with tile.TileContext(nc) as tc:
        with (
            tc.tile_pool(name="const", bufs=1) as const,
            tc.tile_pool(name="xp", bufs=3) as xp,
            tc.tile_pool(name="work", bufs=3) as work,
            tc.tile_pool(name="ep", bufs=4) as ep,
            tc.tile_pool(name="yp", bufs=2) as yp,
            tc.tile_pool(name="ost", bufs=3) as ost,
            tc.tile_pool(name="ps_a", bufs=2, space="PSUM") as ps_a,
            tc.tile_pool(name="ps_s", bufs=2, space="PSUM") as ps_s,
            tc.tile_pool(name="ps_o", bufs=2, space="PSUM") as ps_o,
        ):
            # ---- chunk-0 x as per-slab DMAs so the first qk matmuls can
            # start as soon as slab 0 + the first wqk m-tile land ----
            xts = [xp.tile([128, 8, TCH], BF16, tag="xt", name=f"xt{c}")
                   for c in range(NCH)]
            for s in range(8):
                nc.sync.dma_start(
                    xts[0][:, s, :],
                    xt_in.ap()[s * 128:(s + 1) * 128, 0:TCH])
            # m-tile-major so the first qk matmul only waits on one small DMA
            wqk_sb = const.tile([128, 4, 8, 128], BF16)
            for m in range(4):
                nc.scalar.dma_start(
                    wqk_sb[:, m, :, :],
                    wqk.ap()[:, m * 128:(m + 1) * 128]
                    .rearrange("(s p) c -> p s c", p=128))
            bqk_sb = const.tile([128, 4], F32)
            nc.gpsimd.dma_start(bqk_sb[:], bqk_c[:, :])
            cos_sb = const.tile([128, T], BF16)
            nc.gpsimd.dma_start(cos_sb[:], cos_in[:, :])
            sin_sb = const.tile([128, T], BF16)
            nc.gpsimd.dma_start(sin_sb[:], sin_in[:, :])
            msk_sb = const.tile([128, 128], BF16)
            nc.gpsimd.dma_start(msk_sb[:], cmask[:, :])
            wv_sb = const.tile([128, 8, 256], BF16)
            nc.sync.dma_start(wv_sb[:],
                              wv.ap().rearrange("(s p) m -> p s m", p=128))
            wp_sb = const.tile([128, 2, C], BF16)
            nc.scalar.dma_start(wp_sb[:],
                                wp.ap().rearrange("(s p) m -> p s m", p=128))

            # ---- persistent activations ----
            qT = [const.tile([128, T], BF16, name=f"qT{p}", tag=f"qT{p}")
                  for p in range(2)]
            kT = [const.tile([128, T], BF16, name=f"kT{p}", tag=f"kT{p}")
                  for p in range(2)]
            # v with ones column: [t-slab-part, slab, head, 65]
            v_sb = const.tile([128, NSLAB, HPC, 65], BF16)
            ones128 = const.tile([128, 64], F32)
            nc.gpsimd.memset(ones128[:], 1.0)
            nc.vector.tensor_copy(
                v_sb[:, :, :, 64],
                ones128[:, 0:64].rearrange("p (s h) -> p s h", s=NSLAB))

            swap = [(0, 32, 32, 64), (32, 64, 0, 32),
                    (64, 96, 96, 128), (96, 128, 64, 96)]

            def phase_a_units(tcH):
                """Generator: 8 units (4 qk m-tiles + 4 v tiles) of chunk tcH."""
                tcols = slice(tcH * TCH, (tcH + 1) * TCH)
                xt_ch = xts[tcH]
                if tcH + 1 < NCH:
                    # prefetch next x chunk a full chunk ahead
                    nxt = slice((tcH + 1) * TCH, (tcH + 2) * TCH)
                    nc.sync.dma_start(
                        xts[tcH + 1][:],
                        xt_in.ap()[:, nxt].rearrange("(s p) m -> p s m", p=128))
                for m in range(4):
                    pqk = ps_a.tile([128, TCH], F32, tag="a")
                    for s in range(8):
                        nc.tensor.matmul(pqk[:], wqk_sb[:, m, s, :],
                                         xt_ch[:, s, :], start=(s == 0),
                                         stop=(s == 7))
                    tQr = work.tile([128, TCH], BF16, tag="tQr")
                    nc.vector.tensor_scalar_add(tQr[:], pqk[:],
                                                bqk_sb[:, m:m + 1])
                    tQc = work.tile([128, TCH], BF16, tag="tQc")
                    nc.vector.tensor_mul(tQc[:], tQr[:], cos_sb[:, tcols])
                    tQs = work.tile([128, TCH], BF16, tag="tQs")
                    for (a0, a1, b0, b1) in swap:
                        nc.vector.tensor_mul(tQs[a0:a1, :], tQr[b0:b1, :],
                                             sin_sb[b0:b1, tcols])
                    dest = (qT if m % 2 == 0 else kT)[m // 2]
                    nc.vector.tensor_add(dest[:, tcols], tQc[:], tQs[:])
                    yield
                for ts in range(4):
                    pv = ps_a.tile([128, 256], F32, tag="a")
                    for s in range(8):
                        nc.tensor.matmul(pv[:], xt_ch[:, s, ts * 128:(ts + 1) * 128],
                                         wv_sb[:, s, :], start=(s == 0),
                                         stop=(s == 7))
                    sl = tcH * 4 + ts
                    nc.vector.tensor_copy(
                        v_sb[:, sl, :, 0:64],
                        pv[:].rearrange("p (h e) -> p h e", e=64))
                    yield

            def proj_units(tcH, yT_ch):
                """Generator: 8 units (one out m-tile each) of chunk tcH."""
                tcols = slice(tcH * TCH, (tcH + 1) * TCH)
                for mo in range(8):
                    pp = ps_a.tile([128, TCH], F32, tag="a")
                    for s in range(2):
                        nc.tensor.matmul(pp[:],
                                         wp_sb[:, s, mo * 128:(mo + 1) * 128],
                                         yT_ch[:, s, :], start=(s == 0),
                                         stop=(s == 1))
                    o_st = ost.tile([128, TCH], BF16, tag="ost")
                    nc.vector.tensor_copy(o_st[:], pp[:])
                    nc.sync.dma_start(out_t[mo * 128:(mo + 1) * 128, tcols],
                                      o_st[:])
                    yield

            def attention(tcH, weave):
                """Emit chunk tcH's attention, interleaving `weave` units."""
                yT_ch = yp.tile([128, 2, TCH], BF16, tag="yT")
                nslabs = 4 * tcH + 4
                iters = 2 * nslabs
                # distribute weave units evenly across the j iterations
                acc = [0.0]
                step = weave_len[0] / float(iters) if iters else 0.0

                def advance():
                    acc[0] += step
                    while acc[0] >= 1.0 and weave:
                        try:
                            next(weave[0])
                        except StopIteration:
                            weave.pop(0)
                            continue
                        acc[0] -= 1.0

                for p in range(2):
                    pos = [ps_o.tile([128, TCH], F32, tag="O", name=f"po{hh}")
                           for hh in range(2)]

                    def emit_S(j):
                        rr = j - 4 * tcH
                        r = max(rr, 0) * 128
                        qs = slice(tcH * TCH + r, (tcH + 1) * TCH)
                        psS = ps_s.tile([128, 2, TCH], F32, tag="S")
                        for hh in range(2):
                            base = 64 * hh
                            nc.tensor.matmul(
                                psS[:, hh, r:TCH],
                                kT[p][base:base + 64, j * 128:(j + 1) * 128],
                                qT[p][base:base + 64, qs],
                                start=True, stop=True)
                        expS = ep.tile([128, 2, TCH], BF16, tag="expS")
                        nc.scalar.activation(expS[:, :, r:TCH], psS[:, :, r:TCH],
                                             AF.Exp, scale=0.125)
                        if rr >= 0:
                            for hh in range(2):
                                nc.vector.tensor_mul(expS[:, hh, r:r + 128],
                                                     expS[:, hh, r:r + 128],
                                                     msk_sb[:, :])
                        return expS, r

                    def emit_PV(j, expS, r, hh):
                        h = 2 * p + hh
                        nc.tensor.matmul(pos[hh][0:65, r:TCH],
                                         v_sb[:, j, h, :],
                                         expS[:, hh, r:TCH],
                                         start=(j == 0),
                                         stop=(j == nslabs - 1))

                    def emit_norm(hh):
                        base, po = 64 * hh, pos[hh]
                        l_r = work.tile([1, TCH], F32R, tag="lr")
                        with nc.allow_low_precision(reason="f32r rounding of 1/l"):
                            nc.vector.reciprocal(l_r[:], po[64:65, :])
                        lbc = work.tile([64, TCH], F32R, tag="lbc")
                        nc.gpsimd.partition_broadcast(lbc[:], l_r[:])
                        nc.vector.tensor_mul(yT_ch[base:base + 64, p, :],
                                             po[0:64, :], lbc[:])

                    # software pipeline depth 2: S(j+2) issued before PV(j)
                    win = [emit_S(0)]
                    if nslabs > 1:
                        win.append(emit_S(1))
                    for j in range(nslabs):
                        if j + 2 < nslabs:
                            win.append(emit_S(j + 2))
                        expS, r = win[0]
                        if j == nslabs - 1:
                            emit_PV(j, expS, r, 0)
                            emit_norm(0)
                            emit_PV(j, expS, r, 1)
                            emit_norm(1)
                        else:
                            emit_PV(j, expS, r, 0)
                            emit_PV(j, expS, r, 1)
                        win.pop(0)
                        advance()
                # drain any leftover weave units
                while weave:
                    try:
                        next(weave[0])
                    except StopIteration:
                        weave.pop(0)
                return yT_ch

            # pipeline: weave phase A(t+1) and proj(t-1) into attention(t)
            for _ in phase_a_units(0):
                pass
            yts = {}
            for tcH in range(NCH):
                weave = []
                if tcH >= 1:
                    weave.append(proj_units(tcH - 1, yts[tcH - 1]))
                if tcH + 1 < NCH:
                    weave.append(phase_a_units(tcH + 1))
                weave_len = [8 * len(weave)]
                yts[tcH] = attention(tcH, weave)
            for _ in proj_units(NCH - 1, yts[NCH - 1]):
                pass

    nc.compile()
    return nc


def _rope_tables():
    pos = np.arange(T, dtype=np.float32)[:, None]                  # [T, 1]
    i = np.arange(1, HS // 2 + 1, dtype=np.float32)[None]          # [1, 32]
    theta = 1.0 / 10000.0 ** (2.0 * (i - 1.0) / HS)
    ang = pos * theta
    cos, sin = np.cos(ang).T, np.sin(ang).T                        # [32, T]
    cos_rep = np.tile(cos, (4, 1)).astype(np.float32)              # [128, T]
    sin_sgn = np.concatenate([sin, -sin, sin, -sin], 0).astype(np.float32)
    return cos_rep, sin_sgn


def _mask128():
    p = np.arange(128)[:, None]
    f = np.arange(128)[None, :]
    return (p <= f).astype(np.float32)


def kernel(x, W_qkv, b_qkv, W_proj, b_proj):
    global last_results
    import ml_dtypes
    from concourse.bass_utils import run_bass_kernel_spmd

    bf16 = ml_dtypes.bfloat16

    if "nc" not in _cache:
        _cache["nc"] = _build()
    nc = _cache["nc"]

    x = np.asarray(x, np.float32)
    W_qkv = np.asarray(W_qkv, np.float32)
    b_qkv = np.asarray(b_qkv, np.float32)
    W_proj = np.asarray(W_proj, np.float32)
    b_proj = np.asarray(b_proj, np.float32)

    perm = np.concatenate([np.arange(0, HS, 2), np.arange(1, HS, 2)])  # even|odd
    cos_rep, sin_sgn = _rope_tables()
    cmask = _mask128()

    in_maps = []
    for core in range(NCORES):
        b, g = core // 4, core % 4
        heads = [4 * g + j for j in range(HPC)]
        wq = [W_qkv[:, h * 3 * HS:h * 3 * HS + HS][:, perm] for h in heads]
        wk = [W_qkv[:, h * 3 * HS + HS:h * 3 * HS + 2 * HS][:, perm] for h in heads]
        wv_ = [W_qkv[:, h * 3 * HS + 2 * HS:h * 3 * HS + 3 * HS] for h in heads]
        bq = [b_qkv[h * 3 * HS:h * 3 * HS + HS][perm] for h in heads]
        bk = [b_qkv[h * 3 * HS + HS:h * 3 * HS + 2 * HS][perm] for h in heads]
        # col-chunks: [q01 | k01 | q23 | k23]
        wqk = np.concatenate([wq[0], wq[1], wk[0], wk[1],
                              wq[2], wq[3], wk[2], wk[3]], axis=1)
        bqk = np.concatenate([bq[0], bq[1], bk[0], bk[1],
                              bq[2], bq[3], bk[2], bk[3]])
        in_maps.append({
            "xt_in": np.ascontiguousarray(x[b].T).astype(bf16),
            "wqk": np.ascontiguousarray(wqk).astype(bf16),
            "bqk_c": np.ascontiguousarray(bqk.reshape(4, 128).T),
            "wv": np.ascontiguousarray(np.concatenate(wv_, axis=1)).astype(bf16),
            "wp": np.ascontiguousarray(W_proj[g * 256:(g + 1) * 256, :]).astype(bf16),
            "cos_in": cos_rep.astype(bf16),
            "sin_in": sin_sgn.astype(bf16),
            "cmask": cmask.astype(bf16),
        })

    res = run_bass_kernel_spmd(nc, in_maps, core_ids=list(range(NCORES)))
    last_results = res

    out = np.zeros((B, T, C), dtype=np.float32)
    for core in range(NCORES):
        b = core // 4
        out[b] += res.results[core]["out_t"].astype(np.float32).T
    # v-bias shifts y by exactly bv per head (sum(att) == 1), so its effect
    # on the output is the constant bv_full @ W_proj
    bv_full = np.concatenate(
        [b_qkv[h * 3 * HS + 2 * HS:h * 3 * HS + 3 * HS] for h in range(H)])
    out += (b_proj + bv_full @ W_proj)[None, None, :]
    return out


# revision 36
# speedup vs baseline: 1.1119x; 1.0204x over previous
"""Causal self-attention (B=2, T=2048, C=1024, H=16, rope) on 8 trn2 cores.

Sharding: core i = (batch b = i // 4, head-group g = i % 4 owning heads 4g..4g+3).
Each core computes its 4 heads' attention and a partial projection (transposed,
bf16); the host sums the 4 head-group partials per batch and adds the biases
(v-bias is folded into a host-side constant since sum(att) == 1).

All matmul operands are bf16 (1 cycle/row at any N). x arrives transposed from
the host; rope is 6 tensor ops on DVE (2x bf16 mode); qk bias is a per-partition
tensor_scalar_add fused into the psum evict; exp pairs on Act; causal mask on
DVE; denominators via a ones column in v (PV matmul row 64); 1/l broadcast via
gpsimd partition_broadcast.

Emission WEAVES work at instruction granularity: the attention j-loop of chunk
t is exp(Act)-paced, leaving PE micro-gaps; units of phase A(t+1) (qk m-tiles,
v tiles) and proj(t-1) (mo tiles) are emitted between j iterations so PE's
program order fills those gaps and the normalize chain never blocks proj.
"""

import numpy as np

B, T, C, H = 2, 2048, 1024, 16
HS = C // H            # 64
HPC = H // 4           # 4 heads per core
NCORES = 8
TCH = 512              # t/q chunk size
NCH = T // TCH         # 4 chunks
NSLAB = T // 128       # 16 t-slabs

_cache = {}
last_results = None    # BassKernelResults of the most recent run (for test.py)


def _build():
    import concourse.bacc as bacc
    import concourse.mybir as mybir
    import concourse.tile as tile

    F32 = mybir.dt.float32
    F32R = mybir.dt.float32r
    BF16 = mybir.dt.bfloat16
    AF = mybir.ActivationFunctionType

    nc = bacc.Bacc("TRN2", target_bir_lowering=False, debug=False,
                   num_devices=NCORES)

    xt_in = nc.dram_tensor("xt_in", (C, T), BF16, kind="ExternalInput")
    wqk = nc.dram_tensor("wqk", (C, 512), BF16, kind="ExternalInput")
    bqk_c = nc.dram_tensor("bqk_c", (128, 4), F32, kind="ExternalInput")
    wv = nc.dram_tensor("wv", (C, 256), BF16, kind="ExternalInput")
    wp = nc.dram_tensor("wp", (256, C), BF16, kind="ExternalInput")
    cos_in = nc.dram_tensor("cos_in", (128, T), BF16, kind="ExternalInput")
    sin_in = nc.dram_tensor("sin_in", (128, T), BF16, kind="ExternalInput")
    cmask = nc.dram_tensor("cmask", (128, 128), BF16, kind="ExternalInput")
    out_t = nc.dram_tensor("out_t", (C, T), BF16, kind="ExternalOutput")

    with tile.TileContext(nc) as tc:
        with (
            tc.tile_pool(name="const", bufs=1) as const,
            tc.tile_pool(name="xp", bufs=3) as xp,
            tc.tile_pool(name="work", bufs=3) as work,
            tc.tile_pool(name="ep", bufs=4) as ep,
            tc.tile_pool(name="yp", bufs=2) as yp,
            tc.tile_pool(name="ost", bufs=3) as ost,
            tc.tile_pool(name="ps_a", bufs=2, space="PSUM") as ps_a,
            tc.tile_pool(name="ps_s", bufs=2, space="PSUM") as ps_s,
            tc.tile_pool(name="ps_o", bufs=2, space="PSUM") as ps_o,
        ):
            # ---- chunk-0 x as per-slab DMAs so the first qk matmuls can
            # start as soon as slab 0 + the first wqk m-tile land ----
            xts = [xp.tile([128, 8, TCH], BF16, tag="xt", name=f"xt{c}")
                   for c in range(NCH)]
            for h2 in range(2):
                nc.sync.dma_start(
                    xts[0][:, 4 * h2:4 * h2 + 4, :],
                    xt_in.ap()[512 * h2:512 * h2 + 512, 0:TCH]
                    .rearrange("(s p) m -> p s m", p=128))
            # m-tile-major so the first qk matmul only waits on one small DMA
            wqk_sb = const.tile([128, 4, 8, 128], BF16)
            for m in range(4):
                nc.scalar.dma_start(
                    wqk_sb[:, m, :, :],
                    wqk.ap()[:, m * 128:(m + 1) * 128]
                    .rearrange("(s p) c -> p s c", p=128))
            # cos/sin tables: chunk 0 now, later chunks woven into phase A
            cos_sb = const.tile([128, T], BF16)
            nc.gpsimd.dma_start(cos_sb[:, 0:TCH], cos_in[:, 0:TCH])
            sin_sb = const.tile([128, T], BF16)
            nc.gpsimd.dma_start(sin_sb[:, 0:TCH], sin_in[:, 0:TCH])
            bqk_sb = const.tile([128, 4], F32)
            nc.gpsimd.dma_start(bqk_sb[:], bqk_c[:, :])
            msk_sb = const.tile([128, 128], BF16)
            nc.gpsimd.dma_start(msk_sb[:], cmask[:, :])
            wv_sb = const.tile([128, 8, 256], BF16)
            nc.sync.dma_start(wv_sb[:],
                              wv.ap().rearrange("(s p) m -> p s m", p=128))
            wp_sb = const.tile([128, 2, C], BF16)
            nc.scalar.dma_start(wp_sb[:],
                                wp.ap().rearrange("(s p) m -> p s m", p=128))

            # ---- persistent activations ----
            qT = [const.tile([128, T], BF16, name=f"qT{p}", tag=f"qT{p}")
                  for p in range(2)]
            kT = [const.tile([128, T], BF16, name=f"kT{p}", tag=f"kT{p}")
                  for p in range(2)]
            # v with ones column: [t-slab-part, slab, head, 65]
            v_sb = const.tile([128, NSLAB, HPC, 65], BF16)
            ones128 = const.tile([128, 64], F32)
            nc.gpsimd.memset(ones128[:], 1.0)
            nc.vector.tensor_copy(
                v_sb[:, :, :, 64],
                ones128[:, 0:64].rearrange("p (s h) -> p s h", s=NSLAB))

            swap = [(0, 32, 32, 64), (32, 64, 0, 32),
                    (64, 96, 96, 128), (96, 128, 64, 96)]

            def phase_a_units(tcH):
                """Generator: 8 units (4 qk m-tiles + 4 v tiles) of chunk tcH."""
                tcols = slice(tcH * TCH, (tcH + 1) * TCH)
                xt_ch = xts[tcH]
                if tcH + 1 < NCH:
                    # prefetch next x chunk + its cos/sin slices a chunk ahead
                    nxt = slice((tcH + 1) * TCH, (tcH + 2) * TCH)
                    nc.sync.dma_start(
                        xts[tcH + 1][:],
                        xt_in.ap()[:, nxt].rearrange("(s p) m -> p s m", p=128))
                    nc.gpsimd.dma_start(cos_sb[:, nxt], cos_in[:, nxt])
                    nc.gpsimd.dma_start(sin_sb[:, nxt], sin_in[:, nxt])
                for m in range(4):
                    pqk = ps_a.tile([128, TCH], F32, tag="a")
                    for s in range(8):
                        nc.tensor.matmul(pqk[:], wqk_sb[:, m, s, :],
                                         xt_ch[:, s, :], start=(s == 0),
                                         stop=(s == 7))
                    tQr = work.tile([128, TCH], BF16, tag="tQr")
                    nc.vector.tensor_scalar_add(tQr[:], pqk[:],
                                                bqk_sb[:, m:m + 1])
                    tQc = work.tile([128, TCH], BF16, tag="tQc")
                    nc.vector.tensor_mul(tQc[:], tQr[:], cos_sb[:, tcols])
                    tQs = work.tile([128, TCH], BF16, tag="tQs")
                    for (a0, a1, b0, b1) in swap:
                        nc.vector.tensor_mul(tQs[a0:a1, :], tQr[b0:b1, :],
                                             sin_sb[b0:b1, tcols])
                    dest = (qT if m % 2 == 0 else kT)[m // 2]
                    nc.vector.tensor_add(dest[:, tcols], tQc[:], tQs[:])
                    yield
                for ts in range(4):
                    pv = ps_a.tile([128, 256], F32, tag="a")
                    for s in range(8):
                        nc.tensor.matmul(pv[:], xt_ch[:, s, ts * 128:(ts + 1) * 128],
                                         wv_sb[:, s, :], start=(s == 0),
                                         stop=(s == 7))
                    sl = tcH * 4 + ts
                    nc.vector.tensor_copy(
                        v_sb[:, sl, :, 0:64],
                        pv[:].rearrange("p (h e) -> p h e", e=64))
                    yield

            def proj_units(tcH, yT_ch):
                """Generator: 8 units (one out m-tile each) of chunk tcH."""
                tcols = slice(tcH * TCH, (tcH + 1) * TCH)
                dma_engs = [nc.sync, nc.scalar, nc.gpsimd]
                for mo in range(8):
                    pp = ps_a.tile([128, TCH], F32, tag="a")
                    for s in range(2):
                        nc.tensor.matmul(pp[:],
                                         wp_sb[:, s, mo * 128:(mo + 1) * 128],
                                         yT_ch[:, s, :], start=(s == 0),
                                         stop=(s == 1))
                    o_st = ost.tile([128, TCH], BF16, tag="ost")
                    nc.vector.tensor_copy(o_st[:], pp[:])
                    dma_engs[mo % 3].dma_start(
                        out_t[mo * 128:(mo + 1) * 128, tcols], o_st[:])
                    yield

            def attention(tcH, weave):
                """Emit chunk tcH's attention, interleaving `weave` units."""
                yT_ch = yp.tile([128, 2, TCH], BF16, tag="yT")
                nslabs = 4 * tcH + 4
                iters = 2 * nslabs
                # distribute weave units evenly across the j iterations
                acc = [0.0]
                step = weave_len[0] / float(iters) if iters else 0.0

                def advance():
                    acc[0] += step
                    while acc[0] >= 1.0 and weave:
                        try:
                            next(weave[0])
                        except StopIteration:
                            weave.pop(0)
                            continue
                        acc[0] -= 1.0

                for p in range(2):
                    pos = [ps_o.tile([128, TCH], F32, tag="O", name=f"po{hh}")
                           for hh in range(2)]

                    def emit_S(j):
                        rr = j - 4 * tcH
                        r = max(rr, 0) * 128
                        qs = slice(tcH * TCH + r, (tcH + 1) * TCH)
                        psS = ps_s.tile([128, 2, TCH], F32, tag="S")
                        for hh in range(2):
                            base = 64 * hh
                            nc.tensor.matmul(
                                psS[:, hh, r:TCH],
                                kT[p][base:base + 64, j * 128:(j + 1) * 128],
                                qT[p][base:base + 64, qs],
                                start=True, stop=True)
                        expS = ep.tile([128, 2, TCH], BF16, tag="expS")
                        nc.scalar.activation(expS[:, :, r:TCH], psS[:, :, r:TCH],
                                             AF.Exp, scale=0.125)
                        if rr >= 0:
                            for hh in range(2):
                                nc.vector.tensor_mul(expS[:, hh, r:r + 128],
                                                     expS[:, hh, r:r + 128],
                                                     msk_sb[:, :])
                        return expS, r

                    def emit_PV(j, expS, r, hh):
                        h = 2 * p + hh
                        nc.tensor.matmul(pos[hh][0:65, r:TCH],
                                         v_sb[:, j, h, :],
                                         expS[:, hh, r:TCH],
                                         start=(j == 0),
                                         stop=(j == nslabs - 1))

                    def emit_norm(hh):
                        base, po = 64 * hh, pos[hh]
                        l_r = work.tile([1, TCH], F32R, tag="lr")
                        with nc.allow_low_precision(reason="f32r rounding of 1/l"):
                            nc.vector.reciprocal(l_r[:], po[64:65, :])
                        lbc = work.tile([64, TCH], F32R, tag="lbc")
                        nc.gpsimd.partition_broadcast(lbc[:], l_r[:])
                        nc.vector.tensor_mul(yT_ch[base:base + 64, p, :],
                                             po[0:64, :], lbc[:])

                    # software pipeline depth 2: S(j+2) issued before PV(j)
                    win = [emit_S(0)]
                    if nslabs > 1:
                        win.append(emit_S(1))
                    for j in range(nslabs):
                        if j + 2 < nslabs:
                            win.append(emit_S(j + 2))
                        expS, r = win[0]
                        if j == nslabs - 1:
                            emit_PV(j, expS, r, 0)
                            emit_norm(0)
                            emit_PV(j, expS, r, 1)
                            emit_norm(1)
                        else:
                            emit_PV(j, expS, r, 0)
                            emit_PV(j, expS, r, 1)
                        win.pop(0)
                        advance()
                # drain any leftover weave units
                while weave:
                    try:
                        next(weave[0])
                    except StopIteration:
                        weave.pop(0)
                return yT_ch

            # pipeline: weave phase A(t+1) and proj(t-1) into attention(t)
            for _ in phase_a_units(0):
                pass
            yts = {}
            for tcH in range(NCH):
                weave = []
                if tcH >= 1:
                    weave.append(proj_units(tcH - 1, yts[tcH - 1]))
                if tcH + 1 < NCH:
                    weave.append(phase_a_units(tcH + 1))
                weave_len = [8 * len(weave)]
                yts[tcH] = attention(tcH, weave)
            for _ in proj_units(NCH - 1, yts[NCH - 1]):
                pass

    nc.compile()
    return nc


def _rope_tables():
    pos = np.arange(T, dtype=np.float32)[:, None]                  # [T, 1]
    i = np.arange(1, HS // 2 + 1, dtype=np.float32)[None]          # [1, 32]
    theta = 1.0 / 10000.0 ** (2.0 * (i - 1.0) / HS)
    ang = pos * theta
    cos, sin = np.cos(ang).T, np.sin(ang).T                        # [32, T]
    cos_rep = np.tile(cos, (4, 1)).astype(np.float32)              # [128, T]
    sin_sgn = np.concatenate([sin, -sin, sin, -sin], 0).astype(np.float32)
    return cos_rep, sin_sgn


def _mask128():
    p = np.arange(128)[:, None]
    f = np.arange(128)[None, :]
    return (p <= f).astype(np.float32)


def kernel(x, W_qkv, b_qkv, W_proj, b_proj):
    global last_results
    import ml_dtypes
    from concourse.bass_utils import run_bass_kernel_spmd

    bf16 = ml_dtypes.bfloat16

    if "nc" not in _cache:
        _cache["nc"] = _build()
    nc = _cache["nc"]

    x = np.asarray(x, np.float32)
    W_qkv = np.asarray(W_qkv, np.float32)
    b_qkv = np.asarray(b_qkv, np.float32)
    W_proj = np.asarray(W_proj, np.float32)
    b_proj = np.asarray(b_proj, np.float32)

    perm = np.concatenate([np.arange(0, HS, 2), np.arange(1, HS, 2)])  # even|odd
    cos_rep, sin_sgn = _rope_tables()
    cmask = _mask128()

    in_maps = []
    for core in range(NCORES):
        b, g = core // 4, core % 4
        heads = [4 * g + j for j in range(HPC)]
        wq = [W_qkv[:, h * 3 * HS:h * 3 * HS + HS][:, perm] for h in heads]
        wk = [W_qkv[:, h * 3 * HS + HS:h * 3 * HS + 2 * HS][:, perm] for h in heads]
        wv_ = [W_qkv[:, h * 3 * HS + 2 * HS:h * 3 * HS + 3 * HS] for h in heads]
        bq = [b_qkv[h * 3 * HS:h * 3 * HS + HS][perm] for h in heads]
        bk = [b_qkv[h * 3 * HS + HS:h * 3 * HS + 2 * HS][perm] for h in heads]
        # col-chunks: [q01 | k01 | q23 | k23]
        wqk = np.concatenate([wq[0], wq[1], wk[0], wk[1],
                              wq[2], wq[3], wk[2], wk[3]], axis=1)
        bqk = np.concatenate([bq[0], bq[1], bk[0], bk[1],
                              bq[2], bq[3], bk[2], bk[3]])
        in_maps.append({
            "xt_in": np.ascontiguousarray(x[b].T).astype(bf16),
            "wqk": np.ascontiguousarray(wqk).astype(bf16),
            "bqk_c": np.ascontiguousarray(bqk.reshape(4, 128).T),
            "wv": np.ascontiguousarray(np.concatenate(wv_, axis=1)).astype(bf16),
            "wp": np.ascontiguousarray(W_proj[g * 256:(g + 1) * 256, :]).astype(bf16),
            "cos_in": cos_rep.astype(bf16),
            "sin_in": sin_sgn.astype(bf16),
            "cmask": cmask.astype(bf16),
        })

    res = run_bass_kernel_spmd(nc, in_maps, core_ids=list(range(NCORES)))
    last_results = res

    out = np.zeros((B, T, C), dtype=np.float32)
    for core in range(NCORES):
        b = core // 4
        out[b] += res.results[core]["out_t"].astype(np.float32).T
    # v-bias shifts y by exactly bv per head (sum(att) == 1), so its effect
    # on the output is the constant bv_full @ W_proj
    bv_full = np.concatenate(
        [b_qkv[h * 3 * HS + 2 * HS:h * 3 * HS + 3 * HS] for h in range(H)])
    out += (b_proj + bv_full @ W_proj)[None, None, :]
    return out


# revision 41
# speedup vs baseline: 1.1456x; 1.0303x over previous
"""Causal self-attention (B=2, T=2048, C=1024, H=16, rope) on 8 trn2 cores.

Sharding: core i = (batch b = i // 4, head-group g = i % 4 owning heads 4g..4g+3).
Each core computes its 4 heads' attention and a partial projection (transposed,
bf16); the host sums the 4 head-group partials per batch and adds the biases
(v-bias is folded into a host-side constant since sum(att) == 1).

All matmul operands are bf16 (1 cycle/row at any N). x arrives transposed from
the host; rope is 6 tensor ops on DVE (2x bf16 mode); qk bias is a per-partition
tensor_scalar_add fused into the psum evict; exp pairs on Act; causal mask on
DVE; denominators via a ones column in v (PV matmul row 64); 1/l broadcast via
gpsimd partition_broadcast.

Emission WEAVES work at instruction granularity: the attention j-loop of chunk
t is exp(Act)-paced, leaving PE micro-gaps; units of phase A(t+1) (qk m-tiles,
v tiles) and proj(t-1) (mo tiles) are emitted between j iterations so PE's
program order fills those gaps and the normalize chain never blocks proj.
"""

import numpy as np

B, T, C, H = 2, 2048, 1024, 16
HS = C // H            # 64
HPC = H // 4           # 4 heads per core
NCORES = 8
TCH = 512              # t/q chunk size
NCH = T // TCH         # 4 chunks
NSLAB = T // 128       # 16 t-slabs

_cache = {}
last_results = None    # BassKernelResults of the most recent run (for test.py)


def _build():
    import concourse.bacc as bacc
    import concourse.mybir as mybir
    import concourse.tile as tile

    F32 = mybir.dt.float32
    F32R = mybir.dt.float32r
    BF16 = mybir.dt.bfloat16
    AF = mybir.ActivationFunctionType

    nc = bacc.Bacc("TRN2", target_bir_lowering=False, debug=False,
                   num_devices=NCORES)

    xt_in = nc.dram_tensor("xt_in", (C, T), BF16, kind="ExternalInput")
    wqk = nc.dram_tensor("wqk", (C, 512), BF16, kind="ExternalInput")
    bqk_c = nc.dram_tensor("bqk_c", (128, 4), F32, kind="ExternalInput")
    wv = nc.dram_tensor("wv", (C, 256), BF16, kind="ExternalInput")
    wp = nc.dram_tensor("wp", (256, C), BF16, kind="ExternalInput")
    cos_in = nc.dram_tensor("cos_in", (128, T), BF16, kind="ExternalInput")
    sin_in = nc.dram_tensor("sin_in", (128, T), BF16, kind="ExternalInput")
    cmask = nc.dram_tensor("cmask", (128, 128), BF16, kind="ExternalInput")
    out_t = nc.dram_tensor("out_t", (C, T), BF16, kind="ExternalOutput")

    with tile.TileContext(nc) as tc:
        with (
            tc.tile_pool(name="const", bufs=1) as const,
            tc.tile_pool(name="xp", bufs=3) as xp,
            tc.tile_pool(name="work", bufs=3) as work,
            tc.tile_pool(name="ep", bufs=4) as ep,
            tc.tile_pool(name="yp", bufs=2) as yp,
            tc.tile_pool(name="ost", bufs=3) as ost,
            tc.tile_pool(name="ps_a", bufs=2, space="PSUM") as ps_a,
            tc.tile_pool(name="ps_s", bufs=2, space="PSUM") as ps_s,
            tc.tile_pool(name="ps_o", bufs=2, space="PSUM") as ps_o,
        ):
            # ---- chunk-0 x as per-slab DMAs so the first qk matmuls can
            # start as soon as slab 0 + the first wqk m-tile land ----
            xts = [xp.tile([128, 8, TCH], BF16, tag="xt", name=f"xt{c}")
                   for c in range(NCH)]
            for h2 in range(2):
                nc.sync.dma_start(
                    xts[0][:, 4 * h2:4 * h2 + 4, :],
                    xt_in.ap()[512 * h2:512 * h2 + 512, 0:TCH]
                    .rearrange("(s p) m -> p s m", p=128))
            # m-tile-major so the first qk matmul only waits on one small DMA
            wqk_sb = const.tile([128, 4, 8, 128], BF16)
            for m in range(4):
                nc.scalar.dma_start(
                    wqk_sb[:, m, :, :],
                    wqk.ap()[:, m * 128:(m + 1) * 128]
                    .rearrange("(s p) c -> p s c", p=128))
            # cos/sin tables: chunk 0 now, later chunks woven into phase A
            cos_sb = const.tile([128, T], BF16)
            nc.gpsimd.dma_start(cos_sb[:, 0:TCH], cos_in[:, 0:TCH])
            sin_sb = const.tile([128, T], BF16)
            nc.gpsimd.dma_start(sin_sb[:, 0:TCH], sin_in[:, 0:TCH])
            bqk_sb = const.tile([128, 4], F32)
            nc.gpsimd.dma_start(bqk_sb[:], bqk_c[:, :])
            msk_sb = const.tile([128, 128], BF16)
            nc.gpsimd.dma_start(msk_sb[:], cmask[:, :])
            wv_sb = const.tile([128, 8, 256], BF16)
            nc.sync.dma_start(wv_sb[:],
                              wv.ap().rearrange("(s p) m -> p s m", p=128))
            wp_sb = const.tile([128, 2, C], BF16)
            nc.scalar.dma_start(wp_sb[:],
                                wp.ap().rearrange("(s p) m -> p s m", p=128))

            # ---- persistent activations ----
            qT = [const.tile([128, T], BF16, name=f"qT{p}", tag=f"qT{p}")
                  for p in range(2)]
            kT = [const.tile([128, T], BF16, name=f"kT{p}", tag=f"kT{p}")
                  for p in range(2)]
            # v with ones column: [t-slab-part, slab, head, 65]
            v_sb = const.tile([128, NSLAB, HPC, 65], BF16)
            ones128 = const.tile([128, 64], F32)
            nc.gpsimd.memset(ones128[:], 1.0)
            nc.vector.tensor_copy(
                v_sb[:, :, :, 64],
                ones128[:, 0:64].rearrange("p (s h) -> p s h", s=NSLAB))

            swap = [(0, 32, 32, 64), (32, 64, 0, 32),
                    (64, 96, 96, 128), (96, 128, 64, 96)]

            def phase_a_units(tcH, act_evict):
                """Generator: 8 units (4 qk m-tiles + 4 v tiles) of chunk tcH.
                act_evict routes psum evicts to Act (when the hosting
                attention chunk is PE-paced) or DVE (when Act-exp-paced)."""
                tcols = slice(tcH * TCH, (tcH + 1) * TCH)
                xt_ch = xts[tcH]
                if tcH + 1 < NCH:
                    # prefetch next x chunk + its cos/sin slices a chunk ahead
                    nxt = slice((tcH + 1) * TCH, (tcH + 2) * TCH)
                    nc.sync.dma_start(
                        xts[tcH + 1][:],
                        xt_in.ap()[:, nxt].rearrange("(s p) m -> p s m", p=128))
                    nc.gpsimd.dma_start(cos_sb[:, nxt], cos_in[:, nxt])
                    nc.gpsimd.dma_start(sin_sb[:, nxt], sin_in[:, nxt])
                for m in range(4):
                    pqk = ps_a.tile([128, TCH], F32, tag="a")
                    for s in range(8):
                        nc.tensor.matmul(pqk[:], wqk_sb[:, m, s, :],
                                         xt_ch[:, s, :], start=(s == 0),
                                         stop=(s == 7))
                    tQr = work.tile([128, TCH], BF16, tag="tQr")
                    if act_evict:
                        nc.scalar.add(tQr[:], pqk[:], bqk_sb[:, m:m + 1])
                    else:
                        nc.vector.tensor_scalar_add(tQr[:], pqk[:],
                                                    bqk_sb[:, m:m + 1])
                    tQc = work.tile([128, TCH], BF16, tag="tQc")
                    nc.vector.tensor_mul(tQc[:], tQr[:], cos_sb[:, tcols])
                    tQs = work.tile([128, TCH], BF16, tag="tQs")
                    for (a0, a1, b0, b1) in swap:
                        nc.vector.tensor_mul(tQs[a0:a1, :], tQr[b0:b1, :],
                                             sin_sb[b0:b1, tcols])
                    dest = (qT if m % 2 == 0 else kT)[m // 2]
                    nc.vector.tensor_add(dest[:, tcols], tQc[:], tQs[:])
                    yield
                for ts in range(4):
                    pv = ps_a.tile([128, 256], F32, tag="a")
                    for s in range(8):
                        nc.tensor.matmul(pv[:], xt_ch[:, s, ts * 128:(ts + 1) * 128],
                                         wv_sb[:, s, :], start=(s == 0),
                                         stop=(s == 7))
                    sl = tcH * 4 + ts
                    if act_evict:
                        nc.scalar.activation(
                            v_sb[:, sl, :, 0:64],
                            pv[:].rearrange("p (h e) -> p h e", e=64), AF.Copy)
                    else:
                        nc.vector.tensor_copy(
                            v_sb[:, sl, :, 0:64],
                            pv[:].rearrange("p (h e) -> p h e", e=64))
                    yield

            def proj_units(tcH, yT_ch, evict_mode):
                """Generator: 8 units (one out m-tile each) of chunk tcH.
                evict_mode: 'act', 'dve', or 'alt' (alternate, for the tail)."""
                tcols = slice(tcH * TCH, (tcH + 1) * TCH)
                dma_engs = [nc.sync, nc.scalar, nc.gpsimd]
                for mo in range(8):
                    pp = ps_a.tile([128, TCH], F32, tag="a")
                    for s in range(2):
                        nc.tensor.matmul(pp[:],
                                         wp_sb[:, s, mo * 128:(mo + 1) * 128],
                                         yT_ch[:, s, :], start=(s == 0),
                                         stop=(s == 1))
                    o_st = ost.tile([128, TCH], BF16, tag="ost")
                    use_act = (evict_mode == "act" or
                               (evict_mode == "alt" and mo % 2 == 1))
                    if use_act:
                        nc.scalar.activation(o_st[:], pp[:], AF.Copy)
                    else:
                        nc.vector.tensor_copy(o_st[:], pp[:])
                    dma_engs[mo % 3].dma_start(
                        out_t[mo * 128:(mo + 1) * 128, tcols], o_st[:])
                    yield

            def attention(tcH, weave):
                """Emit chunk tcH's attention, interleaving `weave` units."""
                yT_ch = yp.tile([128, 2, TCH], BF16, tag="yT")
                nslabs = 4 * tcH + 4
                iters = 2 * nslabs
                # distribute weave units evenly across the j iterations
                acc = [0.0]
                step = weave_len[0] / float(iters) if iters else 0.0

                def advance():
                    acc[0] += step
                    while acc[0] >= 1.0 and weave:
                        try:
                            next(weave[0])
                        except StopIteration:
                            weave.pop(0)
                            continue
                        acc[0] -= 1.0

                for p in range(2):
                    pos = [ps_o.tile([128, TCH], F32, tag="O", name=f"po{hh}")
                           for hh in range(2)]

                    def emit_S(j):
                        rr = j - 4 * tcH
                        r = max(rr, 0) * 128
                        qs = slice(tcH * TCH + r, (tcH + 1) * TCH)
                        psS = ps_s.tile([128, 2, TCH], F32, tag="S")
                        for hh in range(2):
                            base = 64 * hh
                            nc.tensor.matmul(
                                psS[:, hh, r:TCH],
                                kT[p][base:base + 64, j * 128:(j + 1) * 128],
                                qT[p][base:base + 64, qs],
                                start=True, stop=True)
                        expS = ep.tile([128, 2, TCH], BF16, tag="expS")
                        nc.scalar.activation(expS[:, :, r:TCH], psS[:, :, r:TCH],
                                             AF.Exp, scale=0.125)
                        if rr >= 0:
                            for hh in range(2):
                                nc.vector.tensor_mul(expS[:, hh, r:r + 128],
                                                     expS[:, hh, r:r + 128],
                                                     msk_sb[:, :])
                        return expS, r

                    def emit_PV(j, expS, r, hh):
                        h = 2 * p + hh
                        nc.tensor.matmul(pos[hh][0:65, r:TCH],
                                         v_sb[:, j, h, :],
                                         expS[:, hh, r:TCH],
                                         start=(j == 0),
                                         stop=(j == nslabs - 1))

                    def emit_norm(hh):
                        base, po = 64 * hh, pos[hh]
                        l_r = work.tile([1, TCH], F32R, tag="lr")
                        with nc.allow_low_precision(reason="f32r rounding of 1/l"):
                            nc.vector.reciprocal(l_r[:], po[64:65, :])
                        lbc = work.tile([64, TCH], F32R, tag="lbc")
                        nc.gpsimd.partition_broadcast(lbc[:], l_r[:])
                        nc.vector.tensor_mul(yT_ch[base:base + 64, p, :],
                                             po[0:64, :], lbc[:])

                    # software pipeline depth 2: S(j+2) issued before PV(j)
                    win = [emit_S(0)]
                    if nslabs > 1:
                        win.append(emit_S(1))
                    for j in range(nslabs):
                        if j + 2 < nslabs:
                            win.append(emit_S(j + 2))
                        expS, r = win[0]
                        if j == nslabs - 1:
                            emit_PV(j, expS, r, 0)
                            emit_norm(0)
                            emit_PV(j, expS, r, 1)
                            emit_norm(1)
                        else:
                            emit_PV(j, expS, r, 0)
                            emit_PV(j, expS, r, 1)
                        win.pop(0)
                        advance()
                # drain any leftover weave units
                while weave:
                    try:
                        next(weave[0])
                    except StopIteration:
                        weave.pop(0)
                return yT_ch

            # pipeline: weave phase A(t+1) and proj(t-1) into attention(t).
            # Chunks 0/1 are PE/DVE-paced -> woven evicts go to Act; chunks
            # 2/3 are Act(exp)-paced -> woven evicts go to DVE.
            for _ in phase_a_units(0, act_evict=False):
                pass
            yts = {}
            for tcH in range(NCH):
                act_side = tcH <= 1
                weave = []
                if tcH >= 1:
                    weave.append(proj_units(tcH - 1, yts[tcH - 1],
                                            "act" if act_side else "dve"))
                if tcH + 1 < NCH:
                    weave.append(phase_a_units(tcH + 1, act_evict=act_side))
                weave_len = [8 * len(weave)]
                yts[tcH] = attention(tcH, weave)
            for _ in proj_units(NCH - 1, yts[NCH - 1], "alt"):
                pass

    nc.compile()
    return nc


def _rope_tables():
    pos = np.arange(T, dtype=np.float32)[:, None]                  # [T, 1]
    i = np.arange(1, HS // 2 + 1, dtype=np.float32)[None]          # [1, 32]
    theta = 1.0 / 10000.0 ** (2.0 * (i - 1.0) / HS)
    ang = pos * theta
    cos, sin = np.cos(ang).T, np.sin(ang).T                        # [32, T]
    cos_rep = np.tile(cos, (4, 1)).astype(np.float32)              # [128, T]
    sin_sgn = np.concatenate([sin, -sin, sin, -sin], 0).astype(np.float32)
    return cos_rep, sin_sgn


def _mask128():
    p = np.arange(128)[:, None]
    f = np.arange(128)[None, :]
    return (p <= f).astype(np.float32)


def kernel(x, W_qkv, b_qkv, W_proj, b_proj):
    global last_results
    import ml_dtypes
    from concourse.bass_utils import run_bass_kernel_spmd

    bf16 = ml_dtypes.bfloat16

    if "nc" not in _cache:
        _cache["nc"] = _build()
    nc = _cache["nc"]

    x = np.asarray(x, np.float32)
    W_qkv = np.asarray(W_qkv, np.float32)
    b_qkv = np.asarray(b_qkv, np.float32)
    W_proj = np.asarray(W_proj, np.float32)
    b_proj = np.asarray(b_proj, np.float32)

    perm = np.concatenate([np.arange(0, HS, 2), np.arange(1, HS, 2)])  # even|odd
    cos_rep, sin_sgn = _rope_tables()
    cmask = _mask128()

    in_maps = []
    for core in range(NCORES):
        b, g = core // 4, core % 4
        heads = [4 * g + j for j in range(HPC)]
        wq = [W_qkv[:, h * 3 * HS:h * 3 * HS + HS][:, perm] for h in heads]
        wk = [W_qkv[:, h * 3 * HS + HS:h * 3 * HS + 2 * HS][:, perm] for h in heads]
        wv_ = [W_qkv[:, h * 3 * HS + 2 * HS:h * 3 * HS + 3 * HS] for h in heads]
        bq = [b_qkv[h * 3 * HS:h * 3 * HS + HS][perm] for h in heads]
        bk = [b_qkv[h * 3 * HS + HS:h * 3 * HS + 2 * HS][perm] for h in heads]
        # col-chunks: [q01 | k01 | q23 | k23]
        wqk = np.concatenate([wq[0], wq[1], wk[0], wk[1],
                              wq[2], wq[3], wk[2], wk[3]], axis=1)
        bqk = np.concatenate([bq[0], bq[1], bk[0], bk[1],
                              bq[2], bq[3], bk[2], bk[3]])
        in_maps.append({
            "xt_in": np.ascontiguousarray(x[b].T).astype(bf16),
            "wqk": np.ascontiguousarray(wqk).astype(bf16),
            "bqk_c": np.ascontiguousarray(bqk.reshape(4, 128).T),
            "wv": np.ascontiguousarray(np.concatenate(wv_, axis=1)).astype(bf16),
            "wp": np.ascontiguousarray(W_proj[g * 256:(g + 1) * 256, :]).astype(bf16),
            "cos_in": cos_rep.astype(bf16),
            "sin_in": sin_sgn.astype(bf16),
            "cmask": cmask.astype(bf16),
        })

    res = run_bass_kernel_spmd(nc, in_maps, core_ids=list(range(NCORES)))
    last_results = res

    out = np.zeros((B, T, C), dtype=np.float32)
    for core in range(NCORES):
        b = core // 4
        out[b] += res.results[core]["out_t"].astype(np.float32).T
    # v-bias shifts y by exactly bv per head (sum(att) == 1), so its effect
    # on the output is the constant bv_full @ W_proj
    bv_full = np.concatenate(
        [b_qkv[h * 3 * HS + 2 * HS:h * 3 * HS + 3 * HS] for h in range(H)])
    out += (b_proj + bv_full @ W_proj)[None, None, :]
    return out


# revision 43
# speedup vs baseline: 1.1492x; 1.0031x over previous
"""Causal self-attention (B=2, T=2048, C=1024, H=16, rope) on 8 trn2 cores.

Sharding: core i = (batch b = i // 4, head-group g = i % 4 owning heads 4g..4g+3).
Each core computes its 4 heads' attention and a partial projection (transposed,
bf16); the host sums the 4 head-group partials per batch and adds the biases
(v-bias is folded into a host-side constant since sum(att) == 1).

All matmul operands are bf16 (1 cycle/row at any N). x arrives transposed from
the host; rope is 6 tensor ops on DVE (2x bf16 mode); qk bias is a per-partition
tensor_scalar_add fused into the psum evict; exp pairs on Act; causal mask on
DVE; denominators via a ones column in v (PV matmul row 64); 1/l broadcast via
gpsimd partition_broadcast.

Emission WEAVES work at instruction granularity: the attention j-loop of chunk
t is exp(Act)-paced, leaving PE micro-gaps; units of phase A(t+1) (qk m-tiles,
v tiles) and proj(t-1) (mo tiles) are emitted between j iterations so PE's
program order fills those gaps and the normalize chain never blocks proj.
"""

import numpy as np

B, T, C, H = 2, 2048, 1024, 16
HS = C // H            # 64
HPC = H // 4           # 4 heads per core
NCORES = 8
TCH = 512              # t/q chunk size
NCH = T // TCH         # 4 chunks
NSLAB = T // 128       # 16 t-slabs

_cache = {}
last_results = None    # BassKernelResults of the most recent run (for test.py)


def _build():
    import concourse.bacc as bacc
    import concourse.mybir as mybir
    import concourse.tile as tile

    F32 = mybir.dt.float32
    F32R = mybir.dt.float32r
    BF16 = mybir.dt.bfloat16
    AF = mybir.ActivationFunctionType

    nc = bacc.Bacc("TRN2", target_bir_lowering=False, debug=False,
                   num_devices=NCORES)

    xt_in = nc.dram_tensor("xt_in", (C, T), BF16, kind="ExternalInput")
    wqk = nc.dram_tensor("wqk", (C, 512), BF16, kind="ExternalInput")
    bqk_c = nc.dram_tensor("bqk_c", (128, 4), F32, kind="ExternalInput")
    wv = nc.dram_tensor("wv", (C, 256), BF16, kind="ExternalInput")
    wp = nc.dram_tensor("wp", (256, C), BF16, kind="ExternalInput")
    cos_in = nc.dram_tensor("cos_in", (128, T), BF16, kind="ExternalInput")
    sin_in = nc.dram_tensor("sin_in", (128, T), BF16, kind="ExternalInput")
    cmask = nc.dram_tensor("cmask", (128, 128), BF16, kind="ExternalInput")
    out_t = nc.dram_tensor("out_t", (C, T), BF16, kind="ExternalOutput")

    with tile.TileContext(nc) as tc:
        with (
            tc.tile_pool(name="const", bufs=1) as const,
            tc.tile_pool(name="xp", bufs=3) as xp,
            tc.tile_pool(name="work", bufs=3) as work,
            tc.tile_pool(name="ep", bufs=4) as ep,
            tc.tile_pool(name="yp", bufs=2) as yp,
            tc.tile_pool(name="ost", bufs=3) as ost,
            tc.tile_pool(name="ps_a", bufs=2, space="PSUM") as ps_a,
            tc.tile_pool(name="ps_s", bufs=2, space="PSUM") as ps_s,
            tc.tile_pool(name="ps_o", bufs=2, space="PSUM") as ps_o,
        ):
            # ---- chunk-0 x as per-slab DMAs so the first qk matmuls can
            # start as soon as slab 0 + the first wqk m-tile land ----
            # m-tile-major wqk; the m0 tile is the very first DMA on the bus
            xts = [xp.tile([128, 8, TCH], BF16, tag="xt", name=f"xt{c}")
                   for c in range(NCH)]
            wqk_sb = const.tile([128, 4, 8, 128], BF16)
            nc.scalar.dma_start(
                wqk_sb[:, 0, :, :],
                wqk.ap()[:, 0:128].rearrange("(s p) c -> p s c", p=128))
            for h2 in range(2):
                nc.sync.dma_start(
                    xts[0][:, 4 * h2:4 * h2 + 4, :],
                    xt_in.ap()[512 * h2:512 * h2 + 512, 0:TCH]
                    .rearrange("(s p) m -> p s m", p=128))
            for m in range(1, 4):
                nc.scalar.dma_start(
                    wqk_sb[:, m, :, :],
                    wqk.ap()[:, m * 128:(m + 1) * 128]
                    .rearrange("(s p) c -> p s c", p=128))
            # cos/sin tables: chunk 0 now, later chunks woven into phase A
            cos_sb = const.tile([128, T], BF16)
            nc.gpsimd.dma_start(cos_sb[:, 0:TCH], cos_in[:, 0:TCH])
            sin_sb = const.tile([128, T], BF16)
            nc.gpsimd.dma_start(sin_sb[:, 0:TCH], sin_in[:, 0:TCH])
            bqk_sb = const.tile([128, 4], F32)
            nc.gpsimd.dma_start(bqk_sb[:], bqk_c[:, :])
            msk_sb = const.tile([128, 128], BF16)
            nc.gpsimd.dma_start(msk_sb[:], cmask[:, :])
            wv_sb = const.tile([128, 8, 256], BF16)
            nc.sync.dma_start(wv_sb[:],
                              wv.ap().rearrange("(s p) m -> p s m", p=128))
            wp_sb = const.tile([128, 2, C], BF16)
            nc.scalar.dma_start(wp_sb[:],
                                wp.ap().rearrange("(s p) m -> p s m", p=128))

            # ---- persistent activations ----
            qT = [const.tile([128, T], BF16, name=f"qT{p}", tag=f"qT{p}")
                  for p in range(2)]
            kT = [const.tile([128, T], BF16, name=f"kT{p}", tag=f"kT{p}")
                  for p in range(2)]
            # v with ones column: [t-slab-part, slab, head, 65]
            v_sb = const.tile([128, NSLAB, HPC, 65], BF16)
            ones128 = const.tile([128, 64], F32)
            nc.gpsimd.memset(ones128[:], 1.0)
            nc.vector.tensor_copy(
                v_sb[:, :, :, 64],
                ones128[:, 0:64].rearrange("p (s h) -> p s h", s=NSLAB))

            swap = [(0, 32, 32, 64), (32, 64, 0, 32),
                    (64, 96, 96, 128), (96, 128, 64, 96)]

            def phase_a_units(tcH, act_evict):
                """Generator: 8 units (4 qk m-tiles + 4 v tiles) of chunk tcH.
                act_evict routes psum evicts to Act (when the hosting
                attention chunk is PE-paced) or DVE (when Act-exp-paced)."""
                tcols = slice(tcH * TCH, (tcH + 1) * TCH)
                xt_ch = xts[tcH]
                if tcH + 1 < NCH:
                    # prefetch next x chunk + its cos/sin slices a chunk ahead
                    nxt = slice((tcH + 1) * TCH, (tcH + 2) * TCH)
                    nc.sync.dma_start(
                        xts[tcH + 1][:],
                        xt_in.ap()[:, nxt].rearrange("(s p) m -> p s m", p=128))
                    nc.gpsimd.dma_start(cos_sb[:, nxt], cos_in[:, nxt])
                    nc.gpsimd.dma_start(sin_sb[:, nxt], sin_in[:, nxt])
                for m in range(4):
                    pqk = ps_a.tile([128, TCH], F32, tag="a")
                    for s in range(8):
                        nc.tensor.matmul(pqk[:], wqk_sb[:, m, s, :],
                                         xt_ch[:, s, :], start=(s == 0),
                                         stop=(s == 7))
                    tQr = work.tile([128, TCH], BF16, tag="tQr")
                    if act_evict:
                        nc.scalar.add(tQr[:], pqk[:], bqk_sb[:, m:m + 1])
                    else:
                        nc.vector.tensor_scalar_add(tQr[:], pqk[:],
                                                    bqk_sb[:, m:m + 1])
                    tQc = work.tile([128, TCH], BF16, tag="tQc")
                    nc.vector.tensor_mul(tQc[:], tQr[:], cos_sb[:, tcols])
                    tQs = work.tile([128, TCH], BF16, tag="tQs")
                    for (a0, a1, b0, b1) in swap:
                        nc.vector.tensor_mul(tQs[a0:a1, :], tQr[b0:b1, :],
                                             sin_sb[b0:b1, tcols])
                    dest = (qT if m % 2 == 0 else kT)[m // 2]
                    nc.vector.tensor_add(dest[:, tcols], tQc[:], tQs[:])
                    yield
                for ts in range(4):
                    pv = ps_a.tile([128, 256], F32, tag="a")
                    for s in range(8):
                        nc.tensor.matmul(pv[:], xt_ch[:, s, ts * 128:(ts + 1) * 128],
                                         wv_sb[:, s, :], start=(s == 0),
                                         stop=(s == 7))
                    sl = tcH * 4 + ts
                    if act_evict:
                        nc.scalar.activation(
                            v_sb[:, sl, :, 0:64],
                            pv[:].rearrange("p (h e) -> p h e", e=64), AF.Copy)
                    else:
                        nc.vector.tensor_copy(
                            v_sb[:, sl, :, 0:64],
                            pv[:].rearrange("p (h e) -> p h e", e=64))
                    yield

            def proj_units(tcH, yT_ch, evict_mode):
                """Generator: 8 units (one out m-tile each) of chunk tcH.
                evict_mode: 'act', 'dve', or 'alt' (alternate, for the tail)."""
                tcols = slice(tcH * TCH, (tcH + 1) * TCH)
                dma_engs = [nc.sync, nc.scalar, nc.gpsimd]
                for mo in range(8):
                    pp = ps_a.tile([128, TCH], F32, tag="a")
                    for s in range(2):
                        nc.tensor.matmul(pp[:],
                                         wp_sb[:, s, mo * 128:(mo + 1) * 128],
                                         yT_ch[:, s, :], start=(s == 0),
                                         stop=(s == 1))
                    o_st = ost.tile([128, TCH], BF16, tag="ost")
                    use_act = (evict_mode == "act" or
                               (evict_mode == "alt" and mo % 2 == 1))
                    if use_act:
                        nc.scalar.activation(o_st[:], pp[:], AF.Copy)
                    else:
                        nc.vector.tensor_copy(o_st[:], pp[:])
                    dma_engs[mo % 3].dma_start(
                        out_t[mo * 128:(mo + 1) * 128, tcols], o_st[:])
                    yield

            def attention(tcH, weave):
                """Emit chunk tcH's attention, interleaving `weave` units."""
                yT_ch = yp.tile([128, 2, TCH], BF16, tag="yT")
                nslabs = 4 * tcH + 4
                iters = 2 * nslabs
                # distribute weave units evenly across the j iterations
                acc = [0.0]
                step = weave_len[0] / float(iters) if iters else 0.0

                def advance():
                    acc[0] += step
                    while acc[0] >= 1.0 and weave:
                        try:
                            next(weave[0])
                        except StopIteration:
                            weave.pop(0)
                            continue
                        acc[0] -= 1.0

                for p in range(2):
                    pos = [ps_o.tile([128, TCH], F32, tag="O", name=f"po{hh}")
                           for hh in range(2)]

                    def emit_S(j):
                        rr = j - 4 * tcH
                        r = max(rr, 0) * 128
                        qs = slice(tcH * TCH + r, (tcH + 1) * TCH)
                        psS = ps_s.tile([128, 2, TCH], F32, tag="S")
                        for hh in range(2):
                            base = 64 * hh
                            nc.tensor.matmul(
                                psS[:, hh, r:TCH],
                                kT[p][base:base + 64, j * 128:(j + 1) * 128],
                                qT[p][base:base + 64, qs],
                                start=True, stop=True)
                        expS = ep.tile([128, 2, TCH], BF16, tag="expS")
                        nc.scalar.activation(expS[:, :, r:TCH], psS[:, :, r:TCH],
                                             AF.Exp, scale=0.125)
                        if rr >= 0:
                            for hh in range(2):
                                nc.vector.tensor_mul(expS[:, hh, r:r + 128],
                                                     expS[:, hh, r:r + 128],
                                                     msk_sb[:, :])
                        return expS, r

                    def emit_PV(j, expS, r, hh):
                        h = 2 * p + hh
                        nc.tensor.matmul(pos[hh][0:65, r:TCH],
                                         v_sb[:, j, h, :],
                                         expS[:, hh, r:TCH],
                                         start=(j == 0),
                                         stop=(j == nslabs - 1))

                    def emit_norm(hh):
                        # stage po out in ONE copy so its psum slot frees
                        # immediately; the recip/broadcast/divide chain then
                        # runs entirely off-psum
                        base, po = 64 * hh, pos[hh]
                        stg = work.tile([65, TCH], F32, tag="stg")
                        nc.vector.tensor_copy(stg[:], po[0:65, :])
                        l_r = work.tile([1, TCH], F32R, tag="lr")
                        with nc.allow_low_precision(reason="f32r rounding of 1/l"):
                            nc.vector.reciprocal(l_r[:], stg[64:65, :])
                        lbc = work.tile([64, TCH], F32R, tag="lbc")
                        nc.gpsimd.partition_broadcast(lbc[:], l_r[:])
                        nc.vector.tensor_mul(yT_ch[base:base + 64, p, :],
                                             stg[0:64, :], lbc[:])

                    # software pipeline depth 2: S(j+2) issued before PV(j)
                    win = [emit_S(0)]
                    if nslabs > 1:
                        win.append(emit_S(1))
                    for j in range(nslabs):
                        if j + 2 < nslabs:
                            win.append(emit_S(j + 2))
                        expS, r = win[0]
                        if j == nslabs - 1:
                            emit_PV(j, expS, r, 0)
                            emit_norm(0)
                            emit_PV(j, expS, r, 1)
                            emit_norm(1)
                        else:
                            emit_PV(j, expS, r, 0)
                            emit_PV(j, expS, r, 1)
                        win.pop(0)
                        advance()
                # drain any leftover weave units
                while weave:
                    try:
                        next(weave[0])
                    except StopIteration:
                        weave.pop(0)
                return yT_ch

            # pipeline: weave phase A(t+1) and proj(t-1) into attention(t).
            # Chunks 0/1 are PE/DVE-paced -> woven evicts go to Act; chunks
            # 2/3 are Act(exp)-paced -> woven evicts go to DVE.
            for _ in phase_a_units(0, act_evict=False):
                pass
            yts = {}
            for tcH in range(NCH):
                act_side = tcH <= 1
                weave = []
                if tcH >= 1:
                    weave.append(proj_units(tcH - 1, yts[tcH - 1],
                                            "act" if act_side else "dve"))
                if tcH + 1 < NCH:
                    weave.append(phase_a_units(tcH + 1, act_evict=act_side))
                weave_len = [8 * len(weave)]
                yts[tcH] = attention(tcH, weave)
            for _ in proj_units(NCH - 1, yts[NCH - 1], "alt"):
                pass

    nc.compile()
    return nc


def _rope_tables():
    pos = np.arange(T, dtype=np.float32)[:, None]                  # [T, 1]
    i = np.arange(1, HS // 2 + 1, dtype=np.float32)[None]          # [1, 32]
    theta = 1.0 / 10000.0 ** (2.0 * (i - 1.0) / HS)
    ang = pos * theta
    cos, sin = np.cos(ang).T, np.sin(ang).T                        # [32, T]
    cos_rep = np.tile(cos, (4, 1)).astype(np.float32)              # [128, T]
    sin_sgn = np.concatenate([sin, -sin, sin, -sin], 0).astype(np.float32)
    return cos_rep, sin_sgn


def _mask128():
    p = np.arange(128)[:, None]
    f = np.arange(128)[None, :]
    return (p <= f).astype(np.float32)


def kernel(x, W_qkv, b_qkv, W_proj, b_proj):
    global last_results
    import ml_dtypes
    from concourse.bass_utils import run_bass_kernel_spmd

    bf16 = ml_dtypes.bfloat16

    if "nc" not in _cache:
        _cache["nc"] = _build()
    nc = _cache["nc"]

    x = np.asarray(x, np.float32)
    W_qkv = np.asarray(W_qkv, np.float32)
    b_qkv = np.asarray(b_qkv, np.float32)
    W_proj = np.asarray(W_proj, np.float32)
    b_proj = np.asarray(b_proj, np.float32)

    perm = np.concatenate([np.arange(0, HS, 2), np.arange(1, HS, 2)])  # even|odd
    cos_rep, sin_sgn = _rope_tables()
    cmask = _mask128()

    in_maps = []
    for core in range(NCORES):
        b, g = core // 4, core % 4
        heads = [4 * g + j for j in range(HPC)]
        wq = [W_qkv[:, h * 3 * HS:h * 3 * HS + HS][:, perm] for h in heads]
        wk = [W_qkv[:, h * 3 * HS + HS:h * 3 * HS + 2 * HS][:, perm] for h in heads]
        wv_ = [W_qkv[:, h * 3 * HS + 2 * HS:h * 3 * HS + 3 * HS] for h in heads]
        bq = [b_qkv[h * 3 * HS:h * 3 * HS + HS][perm] for h in heads]
        bk = [b_qkv[h * 3 * HS + HS:h * 3 * HS + 2 * HS][perm] for h in heads]
        # col-chunks: [q01 | k01 | q23 | k23]
        wqk = np.concatenate([wq[0], wq[1], wk[0], wk[1],
                              wq[2], wq[3], wk[2], wk[3]], axis=1)
        bqk = np.concatenate([bq[0], bq[1], bk[0], bk[1],
                              bq[2], bq[3], bk[2], bk[3]])
        in_maps.append({
            "xt_in": np.ascontiguousarray(x[b].T).astype(bf16),
            "wqk": np.ascontiguousarray(wqk).astype(bf16),
            "bqk_c": np.ascontiguousarray(bqk.reshape(4, 128).T),
            "wv": np.ascontiguousarray(np.concatenate(wv_, axis=1)).astype(bf16),
            "wp": np.ascontiguousarray(W_proj[g * 256:(g + 1) * 256, :]).astype(bf16),
            "cos_in": cos_rep.astype(bf16),
            "sin_in": sin_sgn.astype(bf16),
            "cmask": cmask.astype(bf16),
        })

    res = run_bass_kernel_spmd(nc, in_maps, core_ids=list(range(NCORES)))
    last_results = res

    out = np.zeros((B, T, C), dtype=np.float32)
    for core in range(NCORES):
        b = core // 4
        out[b] += res.results[core]["out_t"].astype(np.float32).T
    # v-bias shifts y by exactly bv per head (sum(att) == 1), so its effect
    # on the output is the constant bv_full @ W_proj
    bv_full = np.concatenate(
        [b_qkv[h * 3 * HS + 2 * HS:h * 3 * HS + 3 * HS] for h in range(H)])
    out += (b_proj + bv_full @ W_proj)[None, None, :]
    return out
